# revision 30
# baseline (speedup 1.0000x reference)
"""LocalMean 5x5 box filter (reflect pad) on TRN2, data-parallel over 8 cores.

Full input:  image (32, 3, 512, 512) fp32
Full output: same shape, 5x5 mean with reflect padding on H and W.

Sharding: batch dim 32 -> 4 images per core (12 channel planes of 512x512).

Final design (each step HW-profiled on this fleet; 641us -> 75us):
  - Single fp16 pipeline (X in [0,1): fp16 round-off ~2^-12; end-to-end
    rel err ~1e-3 incl. fp16 output store, well under the 2e-2 gate).
    This replaces v4's exact bf16+fp16 split, whose gpsimd
    tensor_scalar (+32768 fixed-grid) measured 18.4us/instr here.
  - fp32->fp16 conversion happens INSIDE the load DMAs (SWDGE casts
    inline; HWDGE cannot cast). This removes the DVE cast — the only
    2-port-perf-mode DVE op — which matters because GpSimd/SWDGE
    activity completely stalls DVE 2-port ops (shared SBUF port;
    HW-measured: a 160ns pad copy stretched to 4.4us under a GpSimd
    op). All remaining DVE ops are 1-port fp16 2x_1P.
  - Horizontal 5-tap: A[w] = Xp[w] + Xp[w+2] and P[w] = A[w] + Xp[w+4]
    on DVE (both 4B-aligned fp16 2x packed adds); the remaining taps
    A[w+1] stream into the PE as a second accumulating matmul (the PE
    has no rhs alignment penalty; a DVE op on the misaligned operand
    would drop to 1x mode).
  - Vertical 5-tap via band-matrix matmul (V in {0,1,2}, fp16 exact):
    PSUM = V^T @ P + V^T @ A<<1. M=128 full columns so every PSUM
    partition is written (evac+store never read uninitialized memory).
    2 matmuls per row group x 5 groups = 10 per plane (v4: 30).
  - PSUM evacuation on ScalarE with the 1/25 scale, f16 output;
    groups 3+4 share a 2-bank PSUM tile so they evacuate in one
    ACTIVATE.
  - Output stored as ONE [128, 5*512] f16 DMA per plane (on the Sync
    HWDGE ring — loads own the SWDGE ring, so they overlap) into a
    device-layout tensor; host reassembles rows and upcasts to f32.
    f16 halves store traffic; one DMA per plane keeps queue-issue
    time (~700ns per dma_start) off the critical path.
  - Tail rows (496-511) live in partitions 0-15 of subtile 4 of the
    same tiles, so every elementwise op covers them for free.
"""

import numpy as np

import concourse.bass as bass
import concourse.mybir as mybir
import concourse.tile as tile
from concourse.tile import add_dep_helper
from concourse.bass_utils import run_bass_kernel_spmd

try:
    from bass_rust import AP as RustAP
except ImportError:  # pragma: no cover
    RustAP = None

F32 = mybir.dt.float32
F16 = mybir.dt.float16

N_CORES = 8
NB = 32
NBPC = NB // N_CORES
NCH = NBPC * 3
H = W = 512
PATCH = 5
PAD = 2
INV_AREA = 1.0 / float(PATCH * PATCH)

# Row groups: (in_base, K, out_base, M)
GROUPS = [
    (0, 128, 0, 126),
    (124, 128, 126, 124),
    (248, 128, 250, 124),
    (372, 128, 374, 124),
    (496, 16, 498, 14),
]
XTW = W + 2 * PAD  # 516 padded width
NSUB = 5  # 4 main 128-row subtiles + tail rows in partitions 0-15 of subtile 4


def _reflect(t, n):
    if t < 0:
        t = -t
    if t > n - 1:
        t = 2 * (n - 1) - t
    return t


def _v_matrix(in_base, k_rows, out_base, m_rows):
    v = np.zeros((128, 128), np.float32)
    for m in range(m_rows):
        r = out_base + m
        for t in range(r - PAD, r + PAD + 1):
            k = _reflect(t, H) - in_base
            assert 0 <= k < k_rows, (r, t, k)
            v[k, m] += 1.0
    return v


def _build_vmats():
    v = np.stack(
        [
            _v_matrix(*GROUPS[0]),
            _v_matrix(*GROUPS[1]),
            _v_matrix(*GROUPS[4]),
        ]
    )
    assert np.all(np.isin(v, [0.0, 1.0, 2.0]))
    return v


VMATS16 = _build_vmats().astype(np.float16)
_VM_IDX = [0, 1, 1, 1, 2]


def _mk_ap(like_ap, offset, pattern):
    return RustAP(tensor=like_ap.tensor, offset=offset, ap=pattern)


def build_module(split_waits=True):
    nc = bass.Bass()
    img = nc.dram_tensor("image", [NCH, H, W], F32, kind="ExternalInput")
    vm16 = nc.dram_tensor("vmats16", [3, 128, 128], F16, kind="ExternalInput")
    out1 = nc.dram_tensor(
        "out1", [NCH // 2, 2, 126, 4 * W], F16, kind="ExternalOutput"
    )
    out2 = nc.dram_tensor("out2", [NCH // 2, 2, 14, W], F16, kind="ExternalOutput")

    with tile.TileContext(nc) as tc:
        with (
            tc.tile_pool(name="const", bufs=1) as constp,
            tc.tile_pool(name="xh", bufs=4) as xhp,
            tc.tile_pool(name="psum", bufs=4, space=bass.MemorySpace.PSUM) as psump,
            tc.tile_pool(name="psum2", bufs=2, space=bass.MemorySpace.PSUM) as psump2,
            tc.tile_pool(name="outp", bufs=3) as outp,
        ):
            vt16 = constp.tile([128, 3 * 128], F16)
            vt16r = vt16[:].rearrange("p (i m) -> p i m", i=3)
            nc.sync.dma_start(
                vt16r, _mk_ap(vm16[:], 0, [[128, 128], [128 * 128, 3], [1, 128]])
            )

            # Warmup matmul consumes the weight tile right after its DMA.
            wup_ps = psump.tile([128, 512], F32, tag="pg1")
            warm = nc.tensor.matmul(
                wup_ps[0:1, 0 : 3 * 128],
                vt16[0:128, 0:1],
                vt16[:],
                start=True,
                stop=True,
            )
            prev = {"mm": warm, "dve": None, "act": None, "gps": None}

            def chain(inst, which):
                p = prev[which]
                if p is not None:
                    add_dep_helper(inst.ins, p.ins, sync=False, reason=which)
                prev[which] = inst
                return inst

            def chain_dma(inst):
                return chain(inst, "gps")

            # fp32->fp16 conversion happens INSIDE the load DMA (SWDGE casts
            # inline). This removes the DVE cast — the only 2-port-mode DVE
            # op — so GpSimd/SWDGE SBUF-port interference with DVE 2-port
            # modes (HW-measured: a 160ns pad copy stretched to 4.4us while
            # a GpSimd op ran) cannot bite: all remaining DVE ops are
            # 1-port fp16 2x_1P.
            for pp in range(NCH // 2):
                cc = 2 * pp
                # One set of tiles covers a PAIR of planes: halves per-op
                # dispatch overhead on the DVE and doubles the PE burst
                # length (20 back-to-back matmuls), which keeps HAM at
                # K=8/8 far more reliably than 10-MM bursts.
                xh = xhp.tile([128, 2 * NSUB * XTW], F16, tag="xh")
                xh6 = xh[:].rearrange("p (b a f) -> p b a f", b=2, a=NSUB)

                # Canonical per-subtile casting loads (SWDGE, 16-lane split)
                for b in range(2):
                    for a in range(4):
                        chain_dma(
                            nc.gpsimd.dma_start(
                                xh6[:, b, a, PAD : PAD + W],
                                img[cc + b, 124 * a : 124 * a + 128, :],
                            )
                        )
                    chain_dma(
                        nc.gpsimd.dma_start(
                            xh6[0:16, b, 4, PAD : PAD + W],
                            img[cc + b, H - 16 : H, :],
                        )
                    )

                # Reflect-pad columns on f16: f 0,1 <- f 4,3 ; 514,515 <- 512,511
                chain(
                    nc.vector.tensor_copy(
                        xh6[:, :, :, 0:2], xh6[:, :, :, 4:2:-1]
                    ),
                    "dve",
                )
                chain(
                    nc.vector.tensor_copy(
                        xh6[:, :, :, XTW - 2 : XTW],
                        xh6[:, :, :, XTW - 4 : XTW - 6 : -1],
                    ),
                    "dve",
                )

                # A[w] = Xp[w] + Xp[w+2]: fp16 2x packed (both operands 4B-aligned)
                at = xhp.tile([128, 2 * NSUB * XTW], F16, tag="a")
                a6 = at[:].rearrange("p (b a f) -> p b a f", b=2, a=NSUB)
                chain(
                    nc.vector.tensor_tensor(
                        a6[:, :, :, 0 : XTW - 2],
                        xh6[:, :, :, 0 : XTW - 2],
                        xh6[:, :, :, 2:XTW],
                        mybir.AluOpType.add,
                    ),
                    "dve",
                )

                # P[w] = A[w] + Xp[w+4] (aligned fp16 2x). The remaining taps
                # A[w+1] (= Xp[w+1]+Xp[w+3]) go straight to the PE as a second
                # accumulating matmul — the PE has no rhs alignment penalty.
                pt = xhp.tile([128, 2 * NSUB * XTW], F16, tag="p")
                p6 = pt[:].rearrange("p (b a f) -> p b a f", b=2, a=NSUB)
                chain(
                    nc.vector.tensor_tensor(
                        p6[:, :, :, 0:W],
                        a6[:, :, :, 0:W],
                        xh6[:, :, :, 4 : 4 + W],
                        mybir.AluOpType.add,
                    ),
                    "dve",
                )

                ot = outp.tile([128, 2 * NSUB * W], F16)
                ot6 = ot[:].rearrange("p (b g f) -> p b g f", b=2, g=NSUB)

                for b in range(2):
                    # Groups 3+4 share one 2-bank PSUM tile (disjoint halves)
                    # so their evacuation is a single ScalarE ACTIVATE.
                    pg34 = psump2.tile([128, 2 * W], F32, tag="pg2", name="pg34t")
                    for g in range(5):
                        if g < 3:
                            pg = psump.tile(
                                [128, W], F32, tag="pg1", name="pg1t"
                            )[0:128, :]
                        else:
                            pg = pg34[0:128, (g - 3) * W : (g - 2) * W]
                        vi = _VM_IDX[g]
                        kk = GROUPS[g][1]
                        lh = vt16r[0:kk, vi, 0:128]
                        chain(
                            nc.tensor.matmul(
                                pg, lh, p6[0:kk, b, g, 0:W], start=True, stop=False
                            ),
                            "mm",
                        )
                        chain(
                            nc.tensor.matmul(
                                pg,
                                lh,
                                a6[0:kk, b, g, 1 : 1 + W],
                                start=False,
                                stop=True,
                            ),
                            "mm",
                        )
                        # Evacuate PSUM -> SBUF f16 with 1/25 scale on ScalarE.
                        if g < 3:
                            chain(
                                nc.scalar.mul(ot6[0:128, b, g, :], pg, INV_AREA),
                                "act",
                            )
                    chain(
                        nc.scalar.mul(
                            ot6[0:128, b, 3:5, :], pg34[0:128, :], INV_AREA
                        ),
                        "act",
                    )
                    # Store this plane's chunks as soon as its evacs land
                    # (Sync ring carries only stores, so the extra issues
                    # are free). Skipping partitions >=126 / >=14 trims the
                    # ~20% garbage the 128-partition layout would carry.
                    nc.sync.dma_start(out1[pp, b], ot6[0:126, b, 0:4, :])
                    nc.sync.dma_start(out2[pp, b], ot6[0:14, b, 4, :])

    if split_waits:
        _split_waits(nc)
    return nc


def _split_waits(nc):
    """Walrus legalization: each 64B ISA instruction has ONE sync-wait slot.

    Tile emits instructions with multiple semaphore waits; split the extras
    into standalone InstEventSemaphore sequencer waits (same engine queue,
    immediately before the instruction) which is semantically identical.
    """
    for fn in nc.m.functions:
        for b in fn.blocks:
            insts = b.instructions
            if not any(
                ins.sync_info and len(ins.sync_info.on_wait) > 1 for ins in insts
            ):
                continue
            new = []
            for ins in insts:
                si = ins.sync_info
                if si and len(si.on_wait) > 1:
                    waits = list(si.on_wait)
                    for w in waits[:-1]:
                        ev = mybir.InstEventSemaphore(
                            name=nc.get_next_instruction_name(),
                            engine=ins.engine,
                            ins=[],
                            outs=[],
                        )
                        ev.sync_info = mybir.SyncInfo(on_wait=[w], on_update=[])
                        new.append(ev)
                    si.on_wait = [waits[-1]]
                new.append(ins)
            b.instructions = new


_NC_CACHE = None


def _get_module():
    global _NC_CACHE
    if _NC_CACHE is None:
        _NC_CACHE = build_module()
    return _NC_CACHE


def kernel(image, _trace=False, _trace_kwargs=None):
    image = np.asarray(image)
    assert image.shape == (NB, 3, H, W), image.shape
    in_dtype = image.dtype
    image = np.ascontiguousarray(image.astype(np.float32, copy=False))

    nc = _get_module()
    in_maps = [
        {
            "image": image[i * NBPC : (i + 1) * NBPC].reshape(NCH, H, W),
            "vmats16": VMATS16,
        }
        for i in range(N_CORES)
    ]
    res = run_bass_kernel_spmd(
        nc,
        in_maps,
        list(range(N_CORES)),
        trace=_trace,
        **(_trace_kwargs or {}),
    )
    # Device layout: out1[pair, b, m, g*W + w] holds output row
    # GROUPS[g].out_base + m (g<4) of plane 2*pair + b; out2 the tail group.
    dev1 = np.concatenate(
        [
            np.asarray(res.results[i]["out1"]).reshape(NBPC, 3, 126, 4 * W)
            for i in range(N_CORES)
        ],
        axis=0,
    )
    dev2 = np.concatenate(
        [
            np.asarray(res.results[i]["out2"]).reshape(NBPC, 3, 14, W)
            for i in range(N_CORES)
        ],
        axis=0,
    )
    full = np.empty((NB, 3, H, W), np.float32)
    for g, (_, _, ob, m) in enumerate(GROUPS[:4]):
        full[:, :, ob : ob + m, :] = dev1[:, :, 0:m, g * W : (g + 1) * W].astype(
            np.float32
        )
    ob, m = GROUPS[4][2], GROUPS[4][3]
    full[:, :, ob : ob + m, :] = dev2.astype(np.float32)
    out = full.astype(in_dtype, copy=False)
    if _trace:
        return out, res
    return out


# revision 33
# speedup vs baseline: 1.0005x; 1.0005x over previous
"""LocalMean 5x5 box filter (reflect pad) on TRN2, data-parallel over 8 cores.

Full input:  image (32, 3, 512, 512) fp32
Full output: same shape, 5x5 mean with reflect padding on H and W.

Sharding: batch dim 32 -> 4 images per core (12 channel planes of 512x512).

Final design (each step HW-profiled on this fleet; 641us -> 75us):
  - Single fp16 pipeline (X in [0,1): fp16 round-off ~2^-12; end-to-end
    rel err ~1e-3 incl. fp16 output store, well under the 2e-2 gate).
    This replaces v4's exact bf16+fp16 split, whose gpsimd
    tensor_scalar (+32768 fixed-grid) measured 18.4us/instr here.
  - fp32->fp16 conversion happens INSIDE the load DMAs (SWDGE casts
    inline; HWDGE cannot cast). This removes the DVE cast — the only
    2-port-perf-mode DVE op — which matters because GpSimd/SWDGE
    activity completely stalls DVE 2-port ops (shared SBUF port;
    HW-measured: a 160ns pad copy stretched to 4.4us under a GpSimd
    op). All remaining DVE ops are 1-port fp16 2x_1P.
  - Horizontal 5-tap: A[w] = Xp[w] + Xp[w+2] and P[w] = A[w] + Xp[w+4]
    on DVE (both 4B-aligned fp16 2x packed adds); the remaining taps
    A[w+1] stream into the PE as a second accumulating matmul (the PE
    has no rhs alignment penalty; a DVE op on the misaligned operand
    would drop to 1x mode).
  - Vertical 5-tap via band-matrix matmul (V in {0,1,2}, fp16 exact):
    PSUM = V^T @ P + V^T @ A<<1. M=128 full columns so every PSUM
    partition is written (evac+store never read uninitialized memory).
    2 matmuls per row group x 5 groups = 10 per plane (v4: 30).
  - PSUM evacuation on ScalarE with the 1/25 scale, f16 output;
    groups 3+4 share a 2-bank PSUM tile so they evacuate in one
    ACTIVATE.
  - Output stored as ONE [128, 5*512] f16 DMA per plane (on the Sync
    HWDGE ring — loads own the SWDGE ring, so they overlap) into a
    device-layout tensor; host reassembles rows and upcasts to f32.
    f16 halves store traffic; one DMA per plane keeps queue-issue
    time (~700ns per dma_start) off the critical path.
  - Tail rows (496-511) live in partitions 0-15 of subtile 4 of the
    same tiles, so every elementwise op covers them for free.
"""

import numpy as np

import concourse.bass as bass
import concourse.mybir as mybir
import concourse.tile as tile
from concourse.tile import add_dep_helper
from concourse.bass_utils import run_bass_kernel_spmd

try:
    from bass_rust import AP as RustAP
except ImportError:  # pragma: no cover
    RustAP = None

F32 = mybir.dt.float32
F16 = mybir.dt.float16

N_CORES = 8
NB = 32
NBPC = NB // N_CORES
NCH = NBPC * 3
H = W = 512
PATCH = 5
PAD = 2
INV_AREA = 1.0 / float(PATCH * PATCH)

# Row groups: (in_base, K, out_base, M)
GROUPS = [
    (0, 128, 0, 126),
    (124, 128, 126, 124),
    (248, 128, 250, 124),
    (372, 128, 374, 124),
    (496, 16, 498, 14),
]
XTW = W + 2 * PAD  # 516 padded width
NSUB = 5  # 4 main 128-row subtiles + tail rows in partitions 0-15 of subtile 4


def _reflect(t, n):
    if t < 0:
        t = -t
    if t > n - 1:
        t = 2 * (n - 1) - t
    return t


def _v_matrix(in_base, k_rows, out_base, m_rows):
    v = np.zeros((128, 128), np.float32)
    for m in range(m_rows):
        r = out_base + m
        for t in range(r - PAD, r + PAD + 1):
            k = _reflect(t, H) - in_base
            assert 0 <= k < k_rows, (r, t, k)
            v[k, m] += 1.0
    return v


def _build_vmats():
    v = np.stack(
        [
            _v_matrix(*GROUPS[0]),
            _v_matrix(*GROUPS[1]),
            _v_matrix(*GROUPS[4]),
        ]
    )
    assert np.all(np.isin(v, [0.0, 1.0, 2.0]))
    return v


VMATS16 = _build_vmats().astype(np.float16)
_VM_IDX = [0, 1, 1, 1, 2]


def _mk_ap(like_ap, offset, pattern):
    return RustAP(tensor=like_ap.tensor, offset=offset, ap=pattern)


def build_module(split_waits=True):
    nc = bass.Bass()
    img = nc.dram_tensor("image", [NCH, H, W], F32, kind="ExternalInput")
    vm16 = nc.dram_tensor("vmats16", [3, 128, 128], F16, kind="ExternalInput")
    out = nc.dram_tensor("out", [NCH, 128, NSUB * W], F16, kind="ExternalOutput")

    with tile.TileContext(nc) as tc:
        with (
            tc.tile_pool(name="const", bufs=1) as constp,
            tc.tile_pool(name="xh", bufs=6) as xhp,
            tc.tile_pool(name="psum", bufs=4, space=bass.MemorySpace.PSUM) as psump,
            tc.tile_pool(name="psum2", bufs=2, space=bass.MemorySpace.PSUM) as psump2,
            tc.tile_pool(name="outp", bufs=4) as outp,
        ):
            vt16 = constp.tile([128, 3 * 128], F16)
            vt16r = vt16[:].rearrange("p (i m) -> p i m", i=3)
            nc.sync.dma_start(
                vt16r, _mk_ap(vm16[:], 0, [[128, 128], [128 * 128, 3], [1, 128]])
            )

            # Warmup matmul consumes the weight tile right after its DMA.
            wup_ps = psump.tile([128, 512], F32, tag="pg1")
            warm = nc.tensor.matmul(
                wup_ps[0:1, 0 : 3 * 128],
                vt16[0:128, 0:1],
                vt16[:],
                start=True,
                stop=True,
            )
            prev = {"mm": warm, "dve": None, "act": None, "gps": None}

            def chain(inst, which):
                p = prev[which]
                if p is not None:
                    add_dep_helper(inst.ins, p.ins, sync=False, reason=which)
                prev[which] = inst
                return inst

            def chain_dma(inst):
                return chain(inst, "gps")

            # fp32->fp16 conversion happens INSIDE the load DMA (SWDGE casts
            # inline). This removes the DVE cast — the only 2-port-mode DVE
            # op — so GpSimd/SWDGE SBUF-port interference with DVE 2-port
            # modes (HW-measured: a 160ns pad copy stretched to 4.4us while
            # a GpSimd op ran) cannot bite: all remaining DVE ops are
            # 1-port fp16 2x_1P.
            for c in range(NCH):
                xh = xhp.tile([128, NSUB * XTW], F16, tag="xh")
                xh3 = xh[:].rearrange("p (a f) -> p a f", a=NSUB)

                # Canonical per-subtile casting loads (SWDGE, 16-lane split)
                for a in range(4):
                    chain_dma(
                        nc.gpsimd.dma_start(
                            xh3[:, a, PAD : PAD + W],
                            img[c, 124 * a : 124 * a + 128, :],
                        )
                    )
                chain_dma(
                    nc.gpsimd.dma_start(
                        xh3[0:16, 4, PAD : PAD + W], img[c, H - 16 : H, :]
                    )
                )

                # Reflect-pad columns on f16: f 0,1 <- f 4,3 ; 514,515 <- 512,511
                chain(nc.vector.tensor_copy(xh3[:, :, 0:2], xh3[:, :, 4:2:-1]), "dve")
                chain(
                    nc.vector.tensor_copy(
                        xh3[:, :, XTW - 2 : XTW], xh3[:, :, XTW - 4 : XTW - 6 : -1]
                    ),
                    "dve",
                )

                # A[w] = Xp[w] + Xp[w+2]: fp16 2x packed (both operands 4B-aligned)
                at = xhp.tile([128, NSUB * XTW], F16, tag="a")
                a3 = at[:].rearrange("p (a f) -> p a f", a=NSUB)
                chain(
                    nc.vector.tensor_tensor(
                        a3[:, :, 0 : XTW - 2],
                        xh3[:, :, 0 : XTW - 2],
                        xh3[:, :, 2:XTW],
                        mybir.AluOpType.add,
                    ),
                    "dve",
                )

                # P[w] = A[w] + Xp[w+4] (aligned fp16 2x). The remaining taps
                # A[w+1] (= Xp[w+1]+Xp[w+3]) go straight to the PE as a second
                # accumulating matmul — the PE has no rhs alignment penalty.
                pt = xhp.tile([128, NSUB * XTW], F16, tag="p")
                p3 = pt[:].rearrange("p (a f) -> p a f", a=NSUB)
                chain(
                    nc.vector.tensor_tensor(
                        p3[:, :, 0:W],
                        a3[:, :, 0:W],
                        xh3[:, :, 4 : 4 + W],
                        mybir.AluOpType.add,
                    ),
                    "dve",
                )

                ot = outp.tile([128, NSUB * W], F16)
                ot3 = ot[:].rearrange("p (g f) -> p g f", g=NSUB)

                # Groups 3+4 share one 2-bank PSUM tile (disjoint halves) so
                # their evacuation is a single ScalarE ACTIVATE.
                pg34 = psump2.tile([128, 2 * W], F32, tag="pg2", name="pg34t")
                for g in range(5):
                    if g < 3:
                        pg = psump.tile([128, W], F32, tag="pg1", name="pg1t")[0:128, :]
                    else:
                        pg = pg34[0:128, (g - 3) * W : (g - 2) * W]
                    vi = _VM_IDX[g]
                    kk = GROUPS[g][1]
                    lh = vt16r[0:kk, vi, 0:128]
                    chain(
                        nc.tensor.matmul(
                            pg, lh, p3[0:kk, g, 0:W], start=True, stop=False
                        ),
                        "mm",
                    )
                    chain(
                        nc.tensor.matmul(
                            pg, lh, a3[0:kk, g, 1 : 1 + W], start=False, stop=True
                        ),
                        "mm",
                    )
                    # Evacuate PSUM -> SBUF f16 with the 1/25 scale on ScalarE.
                    if g < 3:
                        chain(
                            nc.scalar.mul(ot3[0:128, g, :], pg, INV_AREA), "act"
                        )
                chain(
                    nc.scalar.mul(ot3[0:128, 3:5, :], pg34[0:128, :], INV_AREA),
                    "act",
                )

                # One store per plane on the now-idle Sync ring.
                nc.sync.dma_start(out[c], ot[:])

    if split_waits:
        _split_waits(nc)
    return nc


def _split_waits(nc):
    """Walrus legalization: each 64B ISA instruction has ONE sync-wait slot.

    Tile emits instructions with multiple semaphore waits; split the extras
    into standalone InstEventSemaphore sequencer waits (same engine queue,
    immediately before the instruction) which is semantically identical.
    """
    for fn in nc.m.functions:
        for b in fn.blocks:
            insts = b.instructions
            if not any(
                ins.sync_info and len(ins.sync_info.on_wait) > 1 for ins in insts
            ):
                continue
            new = []
            for ins in insts:
                si = ins.sync_info
                if si and len(si.on_wait) > 1:
                    waits = list(si.on_wait)
                    for w in waits[:-1]:
                        ev = mybir.InstEventSemaphore(
                            name=nc.get_next_instruction_name(),
                            engine=ins.engine,
                            ins=[],
                            outs=[],
                        )
                        ev.sync_info = mybir.SyncInfo(on_wait=[w], on_update=[])
                        new.append(ev)
                    si.on_wait = [waits[-1]]
                new.append(ins)
            b.instructions = new


_NC_CACHE = None


def _get_module():
    global _NC_CACHE
    if _NC_CACHE is None:
        _NC_CACHE = build_module()
    return _NC_CACHE


def kernel(image, _trace=False, _trace_kwargs=None):
    image = np.asarray(image)
    assert image.shape == (NB, 3, H, W), image.shape
    in_dtype = image.dtype
    image = np.ascontiguousarray(image.astype(np.float32, copy=False))

    nc = _get_module()
    in_maps = [
        {
            "image": image[i * NBPC : (i + 1) * NBPC].reshape(NCH, H, W),
            "vmats16": VMATS16,
        }
        for i in range(N_CORES)
    ]
    res = run_bass_kernel_spmd(
        nc,
        in_maps,
        list(range(N_CORES)),
        trace=_trace,
        **(_trace_kwargs or {}),
    )
    # Device layout: out[c, m, g*W + w] holds output row GROUPS[g].out_base + m.
    dev = np.concatenate(
        [
            np.asarray(res.results[i]["out"]).reshape(NBPC, 3, 128, NSUB * W)
            for i in range(N_CORES)
        ],
        axis=0,
    )
    full = np.empty((NB, 3, H, W), np.float32)
    for g, (_, _, ob, m) in enumerate(GROUPS):
        full[:, :, ob : ob + m, :] = dev[:, :, 0:m, g * W : (g + 1) * W].astype(
            np.float32
        )
    out = full.astype(in_dtype, copy=False)
    if _trace:
        return out, res
    return out


# revision 34
# speedup vs baseline: 1.0556x; 1.0550x over previous
"""LocalMean 5x5 box filter (reflect pad) on TRN2, data-parallel over 8 cores.

Full input:  image (32, 3, 512, 512) fp32
Full output: same shape, 5x5 mean with reflect padding on H and W.

Sharding: batch dim 32 -> 4 images per core (12 channel planes of 512x512).

Final design (each step HW-profiled on this fleet; 641us -> 75us):
  - Single fp16 pipeline (X in [0,1): fp16 round-off ~2^-12; end-to-end
    rel err ~1e-3 incl. fp16 output store, well under the 2e-2 gate).
    This replaces v4's exact bf16+fp16 split, whose gpsimd
    tensor_scalar (+32768 fixed-grid) measured 18.4us/instr here.
  - fp32->fp16 conversion happens INSIDE the load DMAs (SWDGE casts
    inline; HWDGE cannot cast). This removes the DVE cast — the only
    2-port-perf-mode DVE op — which matters because GpSimd/SWDGE
    activity completely stalls DVE 2-port ops (shared SBUF port;
    HW-measured: a 160ns pad copy stretched to 4.4us under a GpSimd
    op). All remaining DVE ops are 1-port fp16 2x_1P.
  - Horizontal 5-tap: A[w] = Xp[w] + Xp[w+2] and P[w] = A[w] + Xp[w+4]
    on DVE (both 4B-aligned fp16 2x packed adds); the remaining taps
    A[w+1] stream into the PE as a second accumulating matmul (the PE
    has no rhs alignment penalty; a DVE op on the misaligned operand
    would drop to 1x mode).
  - Vertical 5-tap via band-matrix matmul (V in {0,1,2}, fp16 exact):
    PSUM = V^T @ P + V^T @ A<<1. M=128 full columns so every PSUM
    partition is written (evac+store never read uninitialized memory).
    2 matmuls per row group x 5 groups = 10 per plane (v4: 30).
  - PSUM evacuation on ScalarE with the 1/25 scale, f16 output;
    groups 3+4 share a 2-bank PSUM tile so they evacuate in one
    ACTIVATE.
  - Output stored as ONE [128, 5*512] f16 DMA per plane (on the Sync
    HWDGE ring — loads own the SWDGE ring, so they overlap) into a
    device-layout tensor; host reassembles rows and upcasts to f32.
    f16 halves store traffic; one DMA per plane keeps queue-issue
    time (~700ns per dma_start) off the critical path.
  - Tail rows (496-511) live in partitions 0-15 of subtile 4 of the
    same tiles, so every elementwise op covers them for free.
"""

import numpy as np

import concourse.bass as bass
import concourse.mybir as mybir
import concourse.tile as tile
from concourse.tile import add_dep_helper
from concourse.bass_utils import run_bass_kernel_spmd

try:
    from bass_rust import AP as RustAP
except ImportError:  # pragma: no cover
    RustAP = None

F32 = mybir.dt.float32
F16 = mybir.dt.float16

N_CORES = 8
NB = 32
NBPC = NB // N_CORES
NCH = NBPC * 3
H = W = 512
PATCH = 5
PAD = 2
INV_AREA = 1.0 / float(PATCH * PATCH)

# Row groups: (in_base, K, out_base, M)
GROUPS = [
    (0, 128, 0, 126),
    (124, 128, 126, 124),
    (248, 128, 250, 124),
    (372, 128, 374, 124),
    (496, 16, 498, 14),
]
XTW = W + 2 * PAD  # 516 padded width
NSUB = 5  # 4 main 128-row subtiles + tail rows in partitions 0-15 of subtile 4


def _reflect(t, n):
    if t < 0:
        t = -t
    if t > n - 1:
        t = 2 * (n - 1) - t
    return t


def _v_matrix(in_base, k_rows, out_base, m_rows):
    v = np.zeros((128, 128), np.float32)
    for m in range(m_rows):
        r = out_base + m
        for t in range(r - PAD, r + PAD + 1):
            k = _reflect(t, H) - in_base
            assert 0 <= k < k_rows, (r, t, k)
            v[k, m] += 1.0
    return v


def _build_vmats():
    v = np.stack(
        [
            _v_matrix(*GROUPS[0]),
            _v_matrix(*GROUPS[1]),
            _v_matrix(*GROUPS[4]),
        ]
    )
    assert np.all(np.isin(v, [0.0, 1.0, 2.0]))
    return v


VMATS16 = _build_vmats().astype(np.float16)
_VM_IDX = [0, 1, 1, 1, 2]


def _mk_ap(like_ap, offset, pattern):
    return RustAP(tensor=like_ap.tensor, offset=offset, ap=pattern)


def build_module(split_waits=True):
    nc = bass.Bass()
    img = nc.dram_tensor("image", [NCH, H, W], F32, kind="ExternalInput")
    vm16 = nc.dram_tensor("vmats16", [3, 128, 128], F16, kind="ExternalInput")
    out1 = nc.dram_tensor("out1", [NCH, 126, 4 * W], F16, kind="ExternalOutput")
    out2 = nc.dram_tensor("out2", [NCH, 14, W], F16, kind="ExternalOutput")

    with tile.TileContext(nc) as tc:
        with (
            tc.tile_pool(name="const", bufs=1) as constp,
            tc.tile_pool(name="xh", bufs=4) as xhp,
            tc.tile_pool(name="psum", bufs=8, space=bass.MemorySpace.PSUM) as psump,
            tc.tile_pool(name="outp", bufs=3) as outp,
        ):
            vt16 = constp.tile([128, 3 * 128], F16)
            vt16r = vt16[:].rearrange("p (i m) -> p i m", i=3)
            nc.sync.dma_start(
                vt16r, _mk_ap(vm16[:], 0, [[128, 128], [128 * 128, 3], [1, 128]])
            )

            # Warmup matmul consumes the weight tile right after its DMA.
            wup_ps = psump.tile([128, 512], F32, tag="pg1")
            warm = nc.tensor.matmul(
                wup_ps[0:1, 0 : 3 * 128],
                vt16[0:128, 0:1],
                vt16[:],
                start=True,
                stop=True,
            )
            prev = {"mm": warm, "dve": None, "act": None, "gps": None}

            def chain(inst, which):
                p = prev[which]
                if p is not None:
                    add_dep_helper(inst.ins, p.ins, sync=False, reason=which)
                prev[which] = inst
                return inst

            def chain_dma(inst):
                return chain(inst, "gps")

            # fp32->fp16 conversion happens INSIDE the load DMA (SWDGE casts
            # inline). This removes the DVE cast — the only 2-port-mode DVE
            # op — so GpSimd/SWDGE SBUF-port interference with DVE 2-port
            # modes (HW-measured: a 160ns pad copy stretched to 4.4us while
            # a GpSimd op ran) cannot bite: all remaining DVE ops are
            # 1-port fp16 2x_1P.
            for c in range(NCH):
                xh = xhp.tile([128, NSUB * XTW], F16, tag="xh")
                xh3 = xh[:].rearrange("p (a f) -> p a f", a=NSUB)

                # Canonical per-subtile casting loads (SWDGE, 16-lane split)
                for a in range(4):
                    chain_dma(
                        nc.gpsimd.dma_start(
                            xh3[:, a, PAD : PAD + W],
                            img[c, 124 * a : 124 * a + 128, :],
                        )
                    )
                chain_dma(
                    nc.gpsimd.dma_start(
                        xh3[0:16, 4, PAD : PAD + W], img[c, H - 16 : H, :]
                    )
                )

                # Reflect-pad columns on f16: f 0,1 <- f 4,3 ; 514,515 <- 512,511
                chain(nc.vector.tensor_copy(xh3[:, :, 0:2], xh3[:, :, 4:2:-1]), "dve")
                chain(
                    nc.vector.tensor_copy(
                        xh3[:, :, XTW - 2 : XTW], xh3[:, :, XTW - 4 : XTW - 6 : -1]
                    ),
                    "dve",
                )

                # A[w] = Xp[w] + Xp[w+2]: fp16 2x packed (both operands 4B-aligned)
                at = xhp.tile([128, NSUB * XTW], F16, tag="a")
                a3 = at[:].rearrange("p (a f) -> p a f", a=NSUB)
                chain(
                    nc.vector.tensor_tensor(
                        a3[:, :, 0 : XTW - 2],
                        xh3[:, :, 0 : XTW - 2],
                        xh3[:, :, 2:XTW],
                        mybir.AluOpType.add,
                    ),
                    "dve",
                )

                # P[w] = A[w] + Xp[w+4] (aligned fp16 2x). The remaining taps
                # A[w+1] (= Xp[w+1]+Xp[w+3]) go straight to the PE as a second
                # accumulating matmul — the PE has no rhs alignment penalty.
                pt = xhp.tile([128, NSUB * XTW], F16, tag="p")
                p3 = pt[:].rearrange("p (a f) -> p a f", a=NSUB)
                chain(
                    nc.vector.tensor_tensor(
                        p3[:, :, 0:W],
                        a3[:, :, 0:W],
                        xh3[:, :, 4 : 4 + W],
                        mybir.AluOpType.add,
                    ),
                    "dve",
                )

                ot = outp.tile([128, NSUB * W], F16)
                ot3 = ot[:].rearrange("p (g f) -> p g f", g=NSUB)

                for g in range(5):
                    pg = psump.tile([128, W], F32, tag="pg1", name="pg1t")
                    vi = _VM_IDX[g]
                    kk = GROUPS[g][1]
                    lh = vt16r[0:kk, vi, 0:128]
                    chain(
                        nc.tensor.matmul(
                            pg[0:128, :], lh, p3[0:kk, g, 0:W], start=True, stop=False
                        ),
                        "mm",
                    )
                    chain(
                        nc.tensor.matmul(
                            pg[0:128, :],
                            lh,
                            a3[0:kk, g, 1 : 1 + W],
                            start=False,
                            stop=True,
                        ),
                        "mm",
                    )
                    # Evacuate PSUM -> SBUF f16 with the 1/25 scale on ScalarE.
                    chain(
                        nc.scalar.mul(ot3[0:128, g, :], pg[0:128, :], INV_AREA),
                        "act",
                    )

                # Compact stores on the store-only Sync ring: [0:126] of the
                # four main chunks + [0:14] of the tail chunk — trims the ~20%
                # garbage partitions a full 128-partition store would carry.
                nc.sync.dma_start(out1[c], ot[0:126, 0 : 4 * W])
                nc.sync.dma_start(out2[c], ot3[0:14, 4, :])

    if split_waits:
        _split_waits(nc)
    return nc


def _split_waits(nc):
    """Walrus legalization: each 64B ISA instruction has ONE sync-wait slot.

    Tile emits instructions with multiple semaphore waits; split the extras
    into standalone InstEventSemaphore sequencer waits (same engine queue,
    immediately before the instruction) which is semantically identical.
    """
    for fn in nc.m.functions:
        for b in fn.blocks:
            insts = b.instructions
            if not any(
                ins.sync_info and len(ins.sync_info.on_wait) > 1 for ins in insts
            ):
                continue
            new = []
            for ins in insts:
                si = ins.sync_info
                if si and len(si.on_wait) > 1:
                    waits = list(si.on_wait)
                    for w in waits[:-1]:
                        ev = mybir.InstEventSemaphore(
                            name=nc.get_next_instruction_name(),
                            engine=ins.engine,
                            ins=[],
                            outs=[],
                        )
                        ev.sync_info = mybir.SyncInfo(on_wait=[w], on_update=[])
                        new.append(ev)
                    si.on_wait = [waits[-1]]
                new.append(ins)
            b.instructions = new


_NC_CACHE = None


def _get_module():
    global _NC_CACHE
    if _NC_CACHE is None:
        _NC_CACHE = build_module()
    return _NC_CACHE


def kernel(image, _trace=False, _trace_kwargs=None):
    image = np.asarray(image)
    assert image.shape == (NB, 3, H, W), image.shape
    in_dtype = image.dtype
    image = np.ascontiguousarray(image.astype(np.float32, copy=False))

    nc = _get_module()
    in_maps = [
        {
            "image": image[i * NBPC : (i + 1) * NBPC].reshape(NCH, H, W),
            "vmats16": VMATS16,
        }
        for i in range(N_CORES)
    ]
    res = run_bass_kernel_spmd(
        nc,
        in_maps,
        list(range(N_CORES)),
        trace=_trace,
        **(_trace_kwargs or {}),
    )
    # Device layout: out1[c, m, g*W + w] holds output row GROUPS[g].out_base + m
    # (g<4); out2 holds the 14-row tail group.
    dev1 = np.concatenate(
        [
            np.asarray(res.results[i]["out1"]).reshape(NBPC, 3, 126, 4 * W)
            for i in range(N_CORES)
        ],
        axis=0,
    )
    dev2 = np.concatenate(
        [
            np.asarray(res.results[i]["out2"]).reshape(NBPC, 3, 14, W)
            for i in range(N_CORES)
        ],
        axis=0,
    )
    full = np.empty((NB, 3, H, W), np.float32)
    for g, (_, _, ob, m) in enumerate(GROUPS[:4]):
        full[:, :, ob : ob + m, :] = dev1[:, :, 0:m, g * W : (g + 1) * W].astype(
            np.float32
        )
    ob, m = GROUPS[4][2], GROUPS[4][3]
    full[:, :, ob : ob + m, :] = dev2.astype(np.float32)
    out = full.astype(in_dtype, copy=False)
    if _trace:
        return out, res
    return out


# revision 35
# speedup vs baseline: 1.0879x; 1.0306x over previous
"""LocalMean 5x5 box filter (reflect pad) on TRN2, data-parallel over 8 cores.

Full input:  image (32, 3, 512, 512) fp32
Full output: same shape, 5x5 mean with reflect padding on H and W.

Sharding: batch dim 32 -> 4 images per core (12 channel planes of 512x512).

Final design (each step HW-profiled on this fleet; 641us -> 75us):
  - Single fp16 pipeline (X in [0,1): fp16 round-off ~2^-12; end-to-end
    rel err ~1e-3 incl. fp16 output store, well under the 2e-2 gate).
    This replaces v4's exact bf16+fp16 split, whose gpsimd
    tensor_scalar (+32768 fixed-grid) measured 18.4us/instr here.
  - fp32->fp16 conversion happens INSIDE the load DMAs (SWDGE casts
    inline; HWDGE cannot cast). This removes the DVE cast — the only
    2-port-perf-mode DVE op — which matters because GpSimd/SWDGE
    activity completely stalls DVE 2-port ops (shared SBUF port;
    HW-measured: a 160ns pad copy stretched to 4.4us under a GpSimd
    op). All remaining DVE ops are 1-port fp16 2x_1P.
  - Horizontal 5-tap: A[w] = Xp[w] + Xp[w+2] and P[w] = A[w] + Xp[w+4]
    on DVE (both 4B-aligned fp16 2x packed adds); the remaining taps
    A[w+1] stream into the PE as a second accumulating matmul (the PE
    has no rhs alignment penalty; a DVE op on the misaligned operand
    would drop to 1x mode).
  - Vertical 5-tap via band-matrix matmul (V in {0,1,2}, fp16 exact):
    PSUM = V^T @ P + V^T @ A<<1. M=128 full columns so every PSUM
    partition is written (evac+store never read uninitialized memory).
    2 matmuls per row group x 5 groups = 10 per plane (v4: 30).
  - PSUM evacuation on ScalarE with the 1/25 scale, f16 output.
  - Stores ride the Sync HWDGE ring, which carries ONLY stores (loads
    own the SWDGE ring, so the two DMA streams overlap; sharing a ring
    head-of-line-blocked loads behind store semaphores, +18us). Two
    compact f16 stores per plane — [0:126] of the four main chunks and
    [0:14] of the tail chunk — trim the ~20% garbage partitions a full
    128-partition store would carry (~1.3MB/core less HBM write
    traffic). Each store depends on exact whole-evacuation outputs
    (partial-range reads of fused evacuations correlated with an
    intermittent correctness failure in an earlier variant). Host
    reassembles rows and upcasts to f32.
  - Tail rows (496-511) live in partitions 0-15 of subtile 4 of the
    same tiles, so every elementwise op covers them for free.
  - With this structure the kernel sits essentially at the HBM floor:
    DMA union busy ~= (12.97MB fp32 reads + 6.5MB f16 writes) / 358
    GB/s; residual +-4us run spread comes from free-running HAM phase.
"""

import numpy as np

import concourse.bass as bass
import concourse.mybir as mybir
import concourse.tile as tile
from concourse.tile import add_dep_helper
from concourse.bass_utils import run_bass_kernel_spmd

try:
    from bass_rust import AP as RustAP
except ImportError:  # pragma: no cover
    RustAP = None

F32 = mybir.dt.float32
F16 = mybir.dt.float16

N_CORES = 8
NB = 32
NBPC = NB // N_CORES
NCH = NBPC * 3
H = W = 512
PATCH = 5
PAD = 2
INV_AREA = 1.0 / float(PATCH * PATCH)

# Row groups: (in_base, K, out_base, M)
GROUPS = [
    (0, 128, 0, 126),
    (124, 128, 126, 124),
    (248, 128, 250, 124),
    (372, 128, 374, 124),
    (496, 16, 498, 14),
]
XTW = W + 2 * PAD  # 516 padded width
NSUB = 5  # 4 main 128-row subtiles + tail rows in partitions 0-15 of subtile 4


def _reflect(t, n):
    if t < 0:
        t = -t
    if t > n - 1:
        t = 2 * (n - 1) - t
    return t


def _v_matrix(in_base, k_rows, out_base, m_rows):
    v = np.zeros((128, 128), np.float32)
    for m in range(m_rows):
        r = out_base + m
        for t in range(r - PAD, r + PAD + 1):
            k = _reflect(t, H) - in_base
            assert 0 <= k < k_rows, (r, t, k)
            v[k, m] += 1.0
    return v


def _build_vmats():
    v = np.stack(
        [
            _v_matrix(*GROUPS[0]),
            _v_matrix(*GROUPS[1]),
            _v_matrix(*GROUPS[4]),
        ]
    )
    assert np.all(np.isin(v, [0.0, 1.0, 2.0]))
    return v


VMATS16 = _build_vmats().astype(np.float16)
_VM_IDX = [0, 1, 1, 1, 2]


def _mk_ap(like_ap, offset, pattern):
    return RustAP(tensor=like_ap.tensor, offset=offset, ap=pattern)


def build_module(split_waits=True):
    nc = bass.Bass()
    img = nc.dram_tensor("image", [NCH, H, W], F32, kind="ExternalInput")
    vm16 = nc.dram_tensor("vmats16", [3, 128, 128], F16, kind="ExternalInput")
    out1 = nc.dram_tensor("out1", [NCH, 126, 4 * W], F16, kind="ExternalOutput")
    out2 = nc.dram_tensor("out2", [NCH, 14, W], F16, kind="ExternalOutput")

    with tile.TileContext(nc) as tc:
        with (
            tc.tile_pool(name="const", bufs=1) as constp,
            tc.tile_pool(name="xh", bufs=4) as xhp,
            tc.tile_pool(name="psum", bufs=8, space=bass.MemorySpace.PSUM) as psump,
            tc.tile_pool(name="outp", bufs=3) as outp,
        ):
            vt16 = constp.tile([128, 3 * 128], F16)
            vt16r = vt16[:].rearrange("p (i m) -> p i m", i=3)
            nc.sync.dma_start(
                vt16r, _mk_ap(vm16[:], 0, [[128, 128], [128 * 128, 3], [1, 128]])
            )

            # Warmup matmul consumes the weight tile right after its DMA.
            wup_ps = psump.tile([128, 512], F32, tag="pg1")
            warm = nc.tensor.matmul(
                wup_ps[0:1, 0 : 3 * 128],
                vt16[0:128, 0:1],
                vt16[:],
                start=True,
                stop=True,
            )
            prev = {"mm": warm, "dve": None, "act": None, "gps": None}

            def chain(inst, which):
                p = prev[which]
                if p is not None:
                    add_dep_helper(inst.ins, p.ins, sync=False, reason=which)
                prev[which] = inst
                return inst

            def chain_dma(inst):
                return chain(inst, "gps")

            # fp32->fp16 conversion happens INSIDE the load DMA (SWDGE casts
            # inline). This removes the DVE cast — the only 2-port-mode DVE
            # op — so GpSimd/SWDGE SBUF-port interference with DVE 2-port
            # modes (HW-measured: a 160ns pad copy stretched to 4.4us while
            # a GpSimd op ran) cannot bite: all remaining DVE ops are
            # 1-port fp16 2x_1P.
            for c in range(NCH):
                xh = xhp.tile([128, NSUB * XTW], F16, tag="xh")
                xh3 = xh[:].rearrange("p (a f) -> p a f", a=NSUB)

                # Canonical per-subtile casting loads (SWDGE, 16-lane split)
                for a in range(4):
                    chain_dma(
                        nc.gpsimd.dma_start(
                            xh3[:, a, PAD : PAD + W],
                            img[c, 124 * a : 124 * a + 128, :],
                        )
                    )
                chain_dma(
                    nc.gpsimd.dma_start(
                        xh3[0:16, 4, PAD : PAD + W], img[c, H - 16 : H, :]
                    )
                )

                # Reflect-pad columns on f16: f 0,1 <- f 4,3 ; 514,515 <- 512,511
                chain(nc.vector.tensor_copy(xh3[:, :, 0:2], xh3[:, :, 4:2:-1]), "dve")
                chain(
                    nc.vector.tensor_copy(
                        xh3[:, :, XTW - 2 : XTW], xh3[:, :, XTW - 4 : XTW - 6 : -1]
                    ),
                    "dve",
                )

                # A[w] = Xp[w] + Xp[w+2]: fp16 2x packed (both operands 4B-aligned)
                at = xhp.tile([128, NSUB * XTW], F16, tag="a")
                a3 = at[:].rearrange("p (a f) -> p a f", a=NSUB)
                chain(
                    nc.vector.tensor_tensor(
                        a3[:, :, 0 : XTW - 2],
                        xh3[:, :, 0 : XTW - 2],
                        xh3[:, :, 2:XTW],
                        mybir.AluOpType.add,
                    ),
                    "dve",
                )

                # P[w] = A[w] + Xp[w+4] (aligned fp16 2x). The remaining taps
                # A[w+1] (= Xp[w+1]+Xp[w+3]) go straight to the PE as a second
                # accumulating matmul — the PE has no rhs alignment penalty.
                pt = xhp.tile([128, NSUB * XTW], F16, tag="p")
                p3 = pt[:].rearrange("p (a f) -> p a f", a=NSUB)
                chain(
                    nc.vector.tensor_tensor(
                        p3[:, :, 0:W],
                        a3[:, :, 0:W],
                        xh3[:, :, 4 : 4 + W],
                        mybir.AluOpType.add,
                    ),
                    "dve",
                )

                ot = outp.tile([128, NSUB * W], F16)
                ot3 = ot[:].rearrange("p (g f) -> p g f", g=NSUB)

                for g in range(5):
                    pg = psump.tile([128, W], F32, tag="pg1", name="pg1t")
                    vi = _VM_IDX[g]
                    kk = GROUPS[g][1]
                    lh = vt16r[0:kk, vi, 0:128]
                    chain(
                        nc.tensor.matmul(
                            pg[0:128, :], lh, p3[0:kk, g, 0:W], start=True, stop=False
                        ),
                        "mm",
                    )
                    chain(
                        nc.tensor.matmul(
                            pg[0:128, :],
                            lh,
                            a3[0:kk, g, 1 : 1 + W],
                            start=False,
                            stop=True,
                        ),
                        "mm",
                    )
                    # Evacuate PSUM -> SBUF f16 with the 1/25 scale on ScalarE.
                    chain(
                        nc.scalar.mul(ot3[0:128, g, :], pg[0:128, :], INV_AREA),
                        "act",
                    )

                # Compact stores on the store-only Sync ring: [0:126] of the
                # four main chunks + [0:14] of the tail chunk — trims the ~20%
                # garbage partitions a full 128-partition store would carry.
                nc.sync.dma_start(out1[c], ot[0:126, 0 : 4 * W])
                nc.sync.dma_start(out2[c], ot3[0:14, 4, :])

    if split_waits:
        _split_waits(nc)
    return nc


def _split_waits(nc):
    """Walrus legalization: each 64B ISA instruction has ONE sync-wait slot.

    Tile emits instructions with multiple semaphore waits; split the extras
    into standalone InstEventSemaphore sequencer waits (same engine queue,
    immediately before the instruction) which is semantically identical.
    """
    for fn in nc.m.functions:
        for b in fn.blocks:
            insts = b.instructions
            if not any(
                ins.sync_info and len(ins.sync_info.on_wait) > 1 for ins in insts
            ):
                continue
            new = []
            for ins in insts:
                si = ins.sync_info
                if si and len(si.on_wait) > 1:
                    waits = list(si.on_wait)
                    for w in waits[:-1]:
                        ev = mybir.InstEventSemaphore(
                            name=nc.get_next_instruction_name(),
                            engine=ins.engine,
                            ins=[],
                            outs=[],
                        )
                        ev.sync_info = mybir.SyncInfo(on_wait=[w], on_update=[])
                        new.append(ev)
                    si.on_wait = [waits[-1]]
                new.append(ins)
            b.instructions = new


_NC_CACHE = None


def _get_module():
    global _NC_CACHE
    if _NC_CACHE is None:
        _NC_CACHE = build_module()
    return _NC_CACHE


def kernel(image, _trace=False, _trace_kwargs=None):
    image = np.asarray(image)
    assert image.shape == (NB, 3, H, W), image.shape
    in_dtype = image.dtype
    image = np.ascontiguousarray(image.astype(np.float32, copy=False))

    nc = _get_module()
    in_maps = [
        {
            "image": image[i * NBPC : (i + 1) * NBPC].reshape(NCH, H, W),
            "vmats16": VMATS16,
        }
        for i in range(N_CORES)
    ]
    res = run_bass_kernel_spmd(
        nc,
        in_maps,
        list(range(N_CORES)),
        trace=_trace,
        **(_trace_kwargs or {}),
    )
    # Device layout: out1[c, m, g*W + w] holds output row GROUPS[g].out_base + m
    # (g<4); out2 holds the 14-row tail group.
    dev1 = np.concatenate(
        [
            np.asarray(res.results[i]["out1"]).reshape(NBPC, 3, 126, 4 * W)
            for i in range(N_CORES)
        ],
        axis=0,
    )
    dev2 = np.concatenate(
        [
            np.asarray(res.results[i]["out2"]).reshape(NBPC, 3, 14, W)
            for i in range(N_CORES)
        ],
        axis=0,
    )
    full = np.empty((NB, 3, H, W), np.float32)
    for g, (_, _, ob, m) in enumerate(GROUPS[:4]):
        full[:, :, ob : ob + m, :] = dev1[:, :, 0:m, g * W : (g + 1) * W].astype(
            np.float32
        )
    ob, m = GROUPS[4][2], GROUPS[4][3]
    full[:, :, ob : ob + m, :] = dev2.astype(np.float32)
    out = full.astype(in_dtype, copy=False)
    if _trace:
        return out, res
    return out
